# revision 37
# baseline (speedup 1.0000x reference)
"""Multi-head self-attention (N=2, S=2048, E=1024, H=16, D=64) on 8 TRN2 cores.

Sharding: tensor-parallel over heads. Each core owns 2 heads (128 channels):
Wq/Wk/Wv split column-wise (output channels), Wo split row-wise (input
channels); x replicated. Each core computes its heads' attention and a
partial output projection; the host sums the 8 partials and adds bo.

Device layout notes:
- Host pre-transposes x -> xT [E, N*S] and weights so every matmul contracts
  over the partition dim with no on-device transposes (except V, transposed
  on the PE so it can serve as the PV stationary operand).
- Scores are computed transposed (S^T[k, q]) so softmax probabilities come
  out with k on partitions, which is exactly the layout the PV matmul wants
  as its moving operand. The softmax denominator comes free by augmenting V
  with a ones column (row 64 of the PV accumulator = sum_k P[k, q]).
- Softmax skips the max-subtraction: scores*0.125 for this problem live in
  [-4, 4], exp cannot overflow, and softmax is shift-invariant.
- The mask enters as a per-partition additive bias on the exp activation
  ((mask-1)*30 -> exp ~ 0 for masked keys). All-ones mask => zero bias.
- Matmul operands are fp16 (true 1 cycle/row on the PE, 1024-wide moving
  operands, fast weight loads); all accumulation stays fp32 in PSUM, and
  the softmax denominators/reciprocals stay fp32, so the end-to-end error
  is ~1e-3 relative.
"""
import sys

sys.path.insert(0, "/opt/trn_rl_repo")

import numpy as np

import concourse.bass as bass
import concourse.mybir as mybir
from concourse.bass_utils import run_bass_kernel_spmd
from concourse.masks import make_identity
from concourse.tile import TileContext

F32 = mybir.dt.float32
F16 = mybir.dt.float16
AF = mybir.ActivationFunctionType

N_CORES = 8
NB, S, E, D = 2, 2048, 1024, 64
NS = NB * S          # 4096 flattened tokens
C = E // N_CORES     # 128 channels (2 heads) per core
HPC = C // D         # 2 heads per core
NKT = S // 128       # 16 key tiles per sequence
QC = 1024            # q-chunk width (PSUM-sized)
NQC = S // QC        # 2 q chunks per sequence


def _split_excess_waits(nc, max_waits=1):
    """This walrus build only accepts one embedded sync-wait per instruction;
    hoist extras into preceding same-engine NoOps."""
    for f in nc.m.functions:
        for bb in f.blocks:
            new_insts = []
            for ins in bb.instructions:
                si = ins.sync_info
                if si is not None and len(si.on_wait) > max_waits:
                    waits = list(si.on_wait)
                    extra, keep = waits[:-max_waits], waits[-max_waits:]
                    for k, w in enumerate(extra):
                        new_insts.append(mybir.InstNoOp(
                            name=f"{ins.name}-ws{k}", engine=ins.engine,
                            sync_info=mybir.SyncInfo(on_wait=[w], on_update=[])))
                    ins.sync_info = mybir.SyncInfo(
                        on_wait=keep, on_update=list(si.on_update))
                new_insts.append(ins)
            bb.instructions = new_insts


def build_program():
    nc = bass.Bass("TRN2", target_bir_lowering=False, debug=False,
                   num_devices=N_CORES)
    xT_d = nc.dram_tensor("xT", [E, NS], F16, kind="ExternalInput").ap()
    wq_d = nc.dram_tensor("wq", [E, C], F16, kind="ExternalInput").ap()
    wk_d = nc.dram_tensor("wk", [E, C], F16, kind="ExternalInput").ap()
    wv_d = nc.dram_tensor("wv", [E, C], F16, kind="ExternalInput").ap()
    wo_d = nc.dram_tensor("wo", [C, E], F16, kind="ExternalInput").ap()
    bq_d = nc.dram_tensor("bq", [C, 1], F32, kind="ExternalInput").ap()
    bk_d = nc.dram_tensor("bk", [C, 1], F32, kind="ExternalInput").ap()
    bv_d = nc.dram_tensor("bv", [C, 1], F32, kind="ExternalInput").ap()
    mb_d = nc.dram_tensor("maskB", [128, NB * NKT], F32,
                          kind="ExternalInput").ap()
    out_d = nc.dram_tensor("out", [NS, E], F32, kind="ExternalOutput").ap()

    with TileContext(nc) as tc:
        with (
            tc.tile_pool(name="sbW", bufs=1) as sbW,
            tc.tile_pool(name="sbBig", bufs=1) as sbBig,
            tc.tile_pool(name="sbX", bufs=16) as sbX,
            tc.tile_pool(name="sbP", bufs=3) as sbP,
            tc.tile_pool(name="sbO", bufs=2) as sbO,
        ):
            # ---- persistent weights / constants ----
            wq_sb = sbW.tile([128, E // 128, C], F16)
            wk_sb = sbW.tile([128, E // 128, C], F16)
            wv_sb = sbW.tile([128, E // 128, C], F16)
            wo_sb = sbW.tile([C, E], F16)
            bq_sb = sbW.tile([C, 1], F32)
            bk_sb = sbW.tile([C, 1], F32)
            bv_sb = sbW.tile([C, 1], F32)
            mb_sb = sbW.tile([128, NB * NKT], F32)
            ident = sbW.tile([128, 128], F32)
            ones_f = sbW.tile([1, 64], F32)
            ones_h = sbW.tile([1, 64], F16)
            onecol = sbW.tile([128, NB * NKT, 1], F32)

            # per-chunk weight loads so the first projection matmuls can
            # start as soon as their stationary chunk lands
            # weights ride the SWDGE queues so the HWDGE queues can start
            # streaming x tiles immediately
            for e in range(E // 128):
                nc.gpsimd.dma_start(out=wq_sb[:, e, :],
                                    in_=wq_d[e * 128:(e + 1) * 128, :])
                nc.gpsimd.dma_start(out=wk_sb[:, e, :],
                                    in_=wk_d[e * 128:(e + 1) * 128, :])
                nc.gpsimd.dma_start(out=wv_sb[:, e, :],
                                    in_=wv_d[e * 128:(e + 1) * 128, :])
            nc.gpsimd.dma_start(out=wo_sb[:], in_=wo_d[:])
            nc.gpsimd.dma_start(out=bq_sb[:], in_=bq_d[:])
            nc.gpsimd.dma_start(out=bk_sb[:], in_=bk_d[:])
            nc.gpsimd.dma_start(out=bv_sb[:], in_=bv_d[:])
            nc.gpsimd.dma_start(out=mb_sb[:], in_=mb_d[:])
            make_identity(nc, ident[:])
            nc.vector.memset(ones_f[:], 1.0)
            nc.scalar.copy(ones_h[:], ones_f[:])
            nc.vector.memset(onecol[:], 1.0)

            # ---- persistent activations ----
            qT_sb = sbBig.tile([C, NS], F16)        # [c, n*s]
            kT_sb = sbBig.tile([C, NS], F16)
            v_aug = [sbBig.tile([128, NB * NKT, D + 1], F16,
                                name=f"vaug{h}", tag=f"vaug{h}")
                     for h in range(HPC)]           # [k, (n,kt), d|1]
            attn_sb = sbBig.tile([C, NS], F16)      # normalized attn output^T

            # ---- phase A: Q/K/V projections (qT/kT/vT = W.T-slice @ xT),
            # with V transposed on the fly into per-head augmented form ----
            with (
                tc.tile_pool(name="psA", bufs=6, space="PSUM") as psA,
                tc.tile_pool(name="psT", bufs=2, space="PSUM") as psT,
            ):
                for st in range(NS // 512):
                    sl = slice(st * 512, (st + 1) * 512)
                    xts = []
                    for e in range(E // 128):
                        xt = sbX.tile([128, 512], F16, tag="xt")
                        nc.sync.dma_start(
                            out=xt[:], in_=xT_d[e * 128:(e + 1) * 128, sl])
                        xts.append(xt)
                    for w_sb, ps_tag in ((wq_sb, "q"), (wk_sb, "k"),
                                         (wv_sb, "v")):
                        ps = psA.tile([C, 512], F32, tag="proj",
                                      name=f"proj_{ps_tag}")
                        for e in range(E // 128):
                            nc.tensor.matmul(ps[:], w_sb[:, e, :], xts[e][:],
                                             start=(e == 0),
                                             stop=(e == E // 128 - 1))
                        if ps_tag == "q":
                            nc.vector.tensor_scalar_add(qT_sb[:, sl], ps[:],
                                                        bq_sb[:, 0:1])
                        elif ps_tag == "k":
                            nc.vector.tensor_scalar_add(kT_sb[:, sl], ps[:],
                                                        bk_sb[:, 0:1])
                        else:
                            vt = sbP.tile([C, 512], F32, tag="vt")
                            nc.vector.tensor_scalar_add(vt[:], ps[:],
                                                        bv_sb[:, 0:1])
                            for k4 in range(4):     # 4 k-tiles per s-tile
                                slot = st * 4 + k4  # == n*NKT + kt
                                tp = psT.tile([128, 128], F32, tag="tp")
                                nc.tensor.transpose(
                                    tp[:], vt[:, k4 * 128:(k4 + 1) * 128],
                                    ident[:])
                                for h in range(HPC):
                                    nc.vector.tensor_copy(
                                        v_aug[h][:, slot, 0:D],
                                        tp[:, h * D:(h + 1) * D])
                for h in range(HPC):
                    nc.vector.tensor_copy(v_aug[h][:, :, D:D + 1], onecol[:])

            # ---- phase C: attention per (batch, head, q-chunk) ----
            # attn_sb first receives UNNORMALIZED PV^T; denominators are
            # collected into denrow and applied in one deferred pass (C2),
            # so the expensive reciprocal never blocks the PSUM pipeline.
            chunks = [(n, h, qc) for n in range(NB) for h in range(HPC)
                      for qc in range(NQC)]
            denrow = sbBig.tile([1, len(chunks) * QC], F32)
            denrow_h = sbBig.tile([1, len(chunks) * QC], F16)

            def chunk_idx(n, h, qc):
                return (n * HPC + h) * NQC + qc

            with tc.tile_pool(name="psC", bufs=1, space="PSUM") as psC:
                # both heads' streams run interleaved per (n, qc) so the PE
                # always has an independent matmul ready while the other
                # head's exp is in flight
                for n in range(NB):
                    for qc in range(NQC):
                        q0 = n * S + qc * QC
                        pv = [psC.tile([D + 1, QC], F32, tag=f"pv{h}",
                                       name=f"pv{h}") for h in range(HPC)]
                        for kt in range(NKT):
                            k0 = n * S + kt * 128
                            pts = []
                            qks = [psC.tile([128, QC], F32, tag=f"qk{h}",
                                            name=f"qk{h}")
                                   for h in range(HPC)]
                            # alternate heads so consecutive matmuls land on
                            # disjoint PE row groups (bases 0/64) and overlap
                            for j in range(QC // 512):
                                for h in range(HPC):
                                    hsl = slice(h * D, (h + 1) * D)
                                    nc.tensor.matmul(
                                        qks[h][:, j * 512:(j + 1) * 512],
                                        kT_sb[hsl, k0:k0 + 128],
                                        qT_sb[hsl, q0 + j * 512:q0 + (j + 1) * 512],
                                        start=True, stop=True)
                            for h in range(HPC):
                                pt = sbP.tile([128, QC], F16, tag=f"pt{h}",
                                              name=f"pt{h}", bufs=3)
                                nc.scalar.activation(
                                    pt[:], qks[h][:], AF.Exp,
                                    bias=mb_sb[:, n * NKT + kt:n * NKT + kt + 1],
                                    scale=0.125)
                                pts.append(pt)
                            for h in range(HPC):
                                for j in range(QC // 512):
                                    nc.tensor.matmul(
                                        pv[h][:, j * 512:(j + 1) * 512],
                                        v_aug[h][:, n * NKT + kt, :],
                                        pts[h][:, j * 512:(j + 1) * 512],
                                        start=(kt == 0), stop=(kt == NKT - 1))
                        for h in range(HPC):
                            hsl = slice(h * D, (h + 1) * D)
                            ci = chunk_idx(n, h, qc)
                            nc.vector.tensor_copy(
                                denrow[0:1, ci * QC:(ci + 1) * QC],
                                pv[h][D:D + 1, :])
                            nc.vector.tensor_copy(attn_sb[hsl, q0:q0 + QC],
                                                  pv[h][0:D, :])

                # ---- phase C2: one cheap reciprocal, then scale in place ----
                nq = len(chunks) * QC               # 8192 denominators
                den128 = sbW.tile([128, nq // 128], F32)
                recip128 = sbW.tile([128, nq // 128], F32)
                recip128h = sbW.tile([128, nq // 128], F16)
                nc.sync.dma_start(out=den128[:], in_=denrow[:])
                nc.vector.reciprocal(recip128[:], den128[:])
                nc.vector.tensor_copy(recip128h[:], recip128[:])
                nc.sync.dma_start(out=denrow_h[:], in_=recip128h[:])
                for ci, (n, h, qc) in enumerate(chunks):
                    hsl = slice(h * D, (h + 1) * D)
                    q0 = n * S + qc * QC
                    bc = psC.tile([128, QC], F32, tag=f"qk{h}", name="bc",
                                  bufs=1)
                    for j in range(QC // 512):
                        nc.tensor.matmul(
                            bc[0:D, j * 512:(j + 1) * 512],
                            ones_h[0:1, 0:D],
                            denrow_h[0:1, ci * QC + j * 512:ci * QC + (j + 1) * 512],
                            start=True, stop=True)
                    nc.vector.tensor_mul(attn_sb[hsl, q0:q0 + QC],
                                         attn_sb[hsl, q0:q0 + QC],
                                         bc[0:D, :])

            # ---- phase D: partial output projection out = attn^T.T @ woT ----
            with tc.tile_pool(name="psD", bufs=4, space="PSUM") as psD:
                for st in range(NS // 128):
                    op = psD.tile([128, E], F32, tag="op")
                    for j in range(E // 512):
                        nc.tensor.matmul(
                            op[:, j * 512:(j + 1) * 512],
                            attn_sb[:, st * 128:(st + 1) * 128],
                            wo_sb[:, j * 512:(j + 1) * 512],
                            start=True, stop=True)
                    ot = sbO.tile([128, E], F32, tag="ot", bufs=4)
                    # alternate copy + DMA engines to pipeline the drain
                    if st % 2 == 0:
                        nc.scalar.copy(ot[:], op[:])
                        nc.sync.dma_start(
                            out=out_d[st * 128:(st + 1) * 128, :], in_=ot[:])
                    else:
                        nc.vector.tensor_copy(ot[:], op[:])
                        nc.scalar.dma_start(
                            out=out_d[st * 128:(st + 1) * 128, :], in_=ot[:])

    _split_excess_waits(nc)
    return nc


_NC_CACHE = None


def _get_program():
    global _NC_CACHE
    if _NC_CACHE is None:
        _NC_CACHE = build_program()
    return _NC_CACHE


def _prep_inputs(x, mask, Wq, bq, Wk, bk, Wv, bv, Wo, bo):
    x = np.asarray(x, np.float32)
    xT = np.ascontiguousarray(x.reshape(NS, E).T.astype(np.float16))
    maskB = (np.asarray(mask, np.float32) - 1.0) * 30.0
    maskB = np.ascontiguousarray(
        maskB.reshape(NB, NKT, 128).transpose(2, 0, 1).reshape(128, NB * NKT))
    in_maps = []
    for i in range(N_CORES):
        c0 = i * C
        csl = slice(c0, c0 + C)
        in_maps.append({
            "xT": xT,
            "wq": np.ascontiguousarray(
                np.asarray(Wq, np.float32)[csl, :].T.astype(np.float16)),
            "wk": np.ascontiguousarray(
                np.asarray(Wk, np.float32)[csl, :].T.astype(np.float16)),
            "wv": np.ascontiguousarray(
                np.asarray(Wv, np.float32)[csl, :].T.astype(np.float16)),
            "wo": np.ascontiguousarray(
                np.asarray(Wo, np.float32)[:, csl].T.astype(np.float16)),
            "bq": np.asarray(bq, np.float32)[csl].reshape(C, 1).copy(),
            "bk": np.asarray(bk, np.float32)[csl].reshape(C, 1).copy(),
            "bv": np.asarray(bv, np.float32)[csl].reshape(C, 1).copy(),
            "maskB": maskB,
        })
    return in_maps


def run(trace=False, tmpdir=None, **inputs):
    """Run on hardware; returns (output, BassKernelResults)."""
    nc = _get_program()
    in_maps = _prep_inputs(**inputs)
    res = run_bass_kernel_spmd(nc, in_maps, list(range(N_CORES)), trace=trace,
                               tmpdir=tmpdir)
    partial = np.zeros((NS, E), np.float32)
    for i in range(N_CORES):
        partial += res.results[i]["out"]
    bo = np.asarray(inputs["bo"], np.float32)
    out = (partial + bo[None, :]).reshape(NB, S, E)
    return out, res


def kernel(**inputs):
    out, _ = run(trace=False, **inputs)
    return out


# revision 38
# speedup vs baseline: 1.0302x; 1.0302x over previous
"""Multi-head self-attention (N=2, S=2048, E=1024, H=16, D=64) on 8 TRN2 cores.

Sharding: tensor-parallel over heads. Each core owns 2 heads (128 channels):
Wq/Wk/Wv split column-wise (output channels), Wo split row-wise (input
channels); x replicated. Each core computes its heads' attention and a
partial output projection; the host sums the 8 partials and adds bo.

Device layout notes:
- Host pre-transposes x -> xT [E, N*S] and weights so every matmul contracts
  over the partition dim with no on-device transposes (except V, transposed
  on the PE so it can serve as the PV stationary operand).
- Scores are computed transposed (S^T[k, q]) so softmax probabilities come
  out with k on partitions, which is exactly the layout the PV matmul wants
  as its moving operand. The softmax denominator comes free by augmenting V
  with a ones column (row 64 of the PV accumulator = sum_k P[k, q]).
- Softmax skips the max-subtraction: scores*0.125 for this problem live in
  [-4, 4], exp cannot overflow, and softmax is shift-invariant.
- The mask enters as a per-partition additive bias on the exp activation
  ((mask-1)*30 -> exp ~ 0 for masked keys). All-ones mask => zero bias.
- Matmul operands are fp16 (true 1 cycle/row on the PE, 1024-wide moving
  operands, fast weight loads); all accumulation stays fp32 in PSUM, and
  the softmax denominators/reciprocals stay fp32, so the end-to-end error
  is ~1e-3 relative.
"""
import sys

sys.path.insert(0, "/opt/trn_rl_repo")

import numpy as np

import concourse.bass as bass
import concourse.mybir as mybir
from concourse.bass_utils import run_bass_kernel_spmd
from concourse.masks import make_identity
from concourse.tile import TileContext

F32 = mybir.dt.float32
F16 = mybir.dt.float16
AF = mybir.ActivationFunctionType

N_CORES = 8
NB, S, E, D = 2, 2048, 1024, 64
NS = NB * S          # 4096 flattened tokens
C = E // N_CORES     # 128 channels (2 heads) per core
HPC = C // D         # 2 heads per core
NKT = S // 128       # 16 key tiles per sequence
QC = 1024            # q-chunk width (PSUM-sized)
NQC = S // QC        # 2 q chunks per sequence


def _split_excess_waits(nc, max_waits=1):
    """This walrus build only accepts one embedded sync-wait per instruction;
    hoist extras into preceding same-engine NoOps."""
    for f in nc.m.functions:
        for bb in f.blocks:
            new_insts = []
            for ins in bb.instructions:
                si = ins.sync_info
                if si is not None and len(si.on_wait) > max_waits:
                    waits = list(si.on_wait)
                    extra, keep = waits[:-max_waits], waits[-max_waits:]
                    for k, w in enumerate(extra):
                        new_insts.append(mybir.InstNoOp(
                            name=f"{ins.name}-ws{k}", engine=ins.engine,
                            sync_info=mybir.SyncInfo(on_wait=[w], on_update=[])))
                    ins.sync_info = mybir.SyncInfo(
                        on_wait=keep, on_update=list(si.on_update))
                new_insts.append(ins)
            bb.instructions = new_insts


def build_program():
    nc = bass.Bass("TRN2", target_bir_lowering=False, debug=False,
                   num_devices=N_CORES)
    xT_d = nc.dram_tensor("xT", [E, NS], F16, kind="ExternalInput").ap()
    wq_d = nc.dram_tensor("wq", [E, C], F16, kind="ExternalInput").ap()
    wk_d = nc.dram_tensor("wk", [E, C], F16, kind="ExternalInput").ap()
    wv_d = nc.dram_tensor("wv", [E, C], F16, kind="ExternalInput").ap()
    wo_d = nc.dram_tensor("wo", [C, E], F16, kind="ExternalInput").ap()
    bq_d = nc.dram_tensor("bq", [C, 1], F32, kind="ExternalInput").ap()
    bk_d = nc.dram_tensor("bk", [C, 1], F32, kind="ExternalInput").ap()
    bv_d = nc.dram_tensor("bv", [C, 1], F32, kind="ExternalInput").ap()
    mb_d = nc.dram_tensor("maskB", [128, NB * NKT], F32,
                          kind="ExternalInput").ap()
    out_d = nc.dram_tensor("out", [NS, E], F32, kind="ExternalOutput").ap()

    with TileContext(nc) as tc:
        with (
            tc.tile_pool(name="sbW", bufs=1) as sbW,
            tc.tile_pool(name="sbBig", bufs=1) as sbBig,
            tc.tile_pool(name="sbX", bufs=16) as sbX,
            tc.tile_pool(name="sbP", bufs=3) as sbP,
            tc.tile_pool(name="sbO", bufs=2) as sbO,
        ):
            # ---- persistent weights / constants ----
            wq_sb = sbW.tile([128, E // 128, C], F16)
            wk_sb = sbW.tile([128, E // 128, C], F16)
            wv_sb = sbW.tile([128, E // 128, C], F16)
            wo_sb = sbW.tile([C, E], F16)
            bq_sb = sbW.tile([C, 1], F32)
            bk_sb = sbW.tile([C, 1], F32)
            bv_sb = sbW.tile([C, 1], F32)
            mb_sb = sbW.tile([128, NB * NKT], F32)
            ident = sbW.tile([128, 128], F32)
            ones_f = sbW.tile([1, 64], F32)
            ones_h = sbW.tile([1, 64], F16)
            onecol = sbW.tile([128, NB * NKT, 1], F32)

            # per-chunk weight loads so the first projection matmuls can
            # start as soon as their stationary chunk lands
            for e in range(E // 128):
                nc.sync.dma_start(out=wq_sb[:, e, :],
                                  in_=wq_d[e * 128:(e + 1) * 128, :])
                nc.sync.dma_start(out=wk_sb[:, e, :],
                                  in_=wk_d[e * 128:(e + 1) * 128, :])
                nc.sync.dma_start(out=wv_sb[:, e, :],
                                  in_=wv_d[e * 128:(e + 1) * 128, :])
            nc.sync.dma_start(out=wo_sb[:], in_=wo_d[:])
            nc.sync.dma_start(out=bq_sb[:], in_=bq_d[:])
            nc.sync.dma_start(out=bk_sb[:], in_=bk_d[:])
            nc.sync.dma_start(out=bv_sb[:], in_=bv_d[:])
            nc.sync.dma_start(out=mb_sb[:], in_=mb_d[:])
            make_identity(nc, ident[:])
            nc.vector.memset(ones_f[:], 1.0)
            nc.scalar.copy(ones_h[:], ones_f[:])
            nc.vector.memset(onecol[:], 1.0)

            # ---- persistent activations ----
            qT_sb = sbBig.tile([C, NS], F16)        # [c, n*s]
            kT_sb = sbBig.tile([C, NS], F16)
            v_aug = [sbBig.tile([128, NB * NKT, D + 1], F16,
                                name=f"vaug{h}", tag=f"vaug{h}")
                     for h in range(HPC)]           # [k, (n,kt), d|1]
            attn_sb = sbBig.tile([C, NS], F16)      # normalized attn output^T

            # ---- phase A: Q/K/V projections (qT/kT/vT = W.T-slice @ xT),
            # with V transposed on the fly into per-head augmented form ----
            with (
                tc.tile_pool(name="psA", bufs=6, space="PSUM") as psA,
                tc.tile_pool(name="psT", bufs=2, space="PSUM") as psT,
            ):
                for st in range(NS // 512):
                    sl = slice(st * 512, (st + 1) * 512)
                    xts = []
                    for e in range(E // 128):
                        xt = sbX.tile([128, 512], F16, tag="xt")
                        nc.sync.dma_start(
                            out=xt[:], in_=xT_d[e * 128:(e + 1) * 128, sl])
                        xts.append(xt)
                    for w_sb, ps_tag in ((wq_sb, "q"), (wk_sb, "k"),
                                         (wv_sb, "v")):
                        ps = psA.tile([C, 512], F32, tag="proj",
                                      name=f"proj_{ps_tag}")
                        for e in range(E // 128):
                            nc.tensor.matmul(ps[:], w_sb[:, e, :], xts[e][:],
                                             start=(e == 0),
                                             stop=(e == E // 128 - 1))
                        if ps_tag == "q":
                            nc.vector.tensor_scalar_add(qT_sb[:, sl], ps[:],
                                                        bq_sb[:, 0:1])
                        elif ps_tag == "k":
                            nc.vector.tensor_scalar_add(kT_sb[:, sl], ps[:],
                                                        bk_sb[:, 0:1])
                        else:
                            vt = sbP.tile([C, 512], F32, tag="vt")
                            nc.vector.tensor_scalar_add(vt[:], ps[:],
                                                        bv_sb[:, 0:1])
                            for k4 in range(4):     # 4 k-tiles per s-tile
                                slot = st * 4 + k4  # == n*NKT + kt
                                tp = psT.tile([128, 128], F32, tag="tp")
                                nc.tensor.transpose(
                                    tp[:], vt[:, k4 * 128:(k4 + 1) * 128],
                                    ident[:])
                                for h in range(HPC):
                                    nc.vector.tensor_copy(
                                        v_aug[h][:, slot, 0:D],
                                        tp[:, h * D:(h + 1) * D])
                for h in range(HPC):
                    nc.vector.tensor_copy(v_aug[h][:, :, D:D + 1], onecol[:])

            # ---- phase C: attention per (batch, head, q-chunk) ----
            # attn_sb first receives UNNORMALIZED PV^T; denominators are
            # collected into denrow and applied in one deferred pass (C2),
            # so the expensive reciprocal never blocks the PSUM pipeline.
            chunks = [(n, h, qc) for n in range(NB) for h in range(HPC)
                      for qc in range(NQC)]
            denrow = sbBig.tile([1, len(chunks) * QC], F32)
            denrow_h = sbBig.tile([1, len(chunks) * QC], F16)

            def chunk_idx(n, h, qc):
                return (n * HPC + h) * NQC + qc

            with tc.tile_pool(name="psC", bufs=1, space="PSUM") as psC:
                # both heads' streams run interleaved per (n, qc) so the PE
                # always has an independent matmul ready while the other
                # head's exp is in flight
                for n in range(NB):
                    for qc in range(NQC):
                        q0 = n * S + qc * QC
                        pv = [psC.tile([D + 1, QC], F32, tag=f"pv{h}",
                                       name=f"pv{h}") for h in range(HPC)]
                        for kt in range(NKT):
                            k0 = n * S + kt * 128
                            pts = []
                            qks = [psC.tile([128, QC], F32, tag=f"qk{h}",
                                            name=f"qk{h}")
                                   for h in range(HPC)]
                            # alternate heads so consecutive matmuls land on
                            # disjoint PE row groups (bases 0/64) and overlap
                            for j in range(QC // 512):
                                for h in range(HPC):
                                    hsl = slice(h * D, (h + 1) * D)
                                    nc.tensor.matmul(
                                        qks[h][:, j * 512:(j + 1) * 512],
                                        kT_sb[hsl, k0:k0 + 128],
                                        qT_sb[hsl, q0 + j * 512:q0 + (j + 1) * 512],
                                        start=True, stop=True)
                            for h in range(HPC):
                                pt = sbP.tile([128, QC], F16, tag=f"pt{h}",
                                              name=f"pt{h}", bufs=3)
                                nc.scalar.activation(
                                    pt[:], qks[h][:], AF.Exp,
                                    bias=mb_sb[:, n * NKT + kt:n * NKT + kt + 1],
                                    scale=0.125)
                                pts.append(pt)
                            for h in range(HPC):
                                for j in range(QC // 512):
                                    nc.tensor.matmul(
                                        pv[h][:, j * 512:(j + 1) * 512],
                                        v_aug[h][:, n * NKT + kt, :],
                                        pts[h][:, j * 512:(j + 1) * 512],
                                        start=(kt == 0), stop=(kt == NKT - 1))
                        for h in range(HPC):
                            hsl = slice(h * D, (h + 1) * D)
                            ci = chunk_idx(n, h, qc)
                            nc.vector.tensor_copy(
                                denrow[0:1, ci * QC:(ci + 1) * QC],
                                pv[h][D:D + 1, :])
                            nc.vector.tensor_copy(attn_sb[hsl, q0:q0 + QC],
                                                  pv[h][0:D, :])

                # ---- phase C2: one cheap reciprocal, then scale in place ----
                nq = len(chunks) * QC               # 8192 denominators
                den128 = sbW.tile([128, nq // 128], F32)
                recip128 = sbW.tile([128, nq // 128], F32)
                recip128h = sbW.tile([128, nq // 128], F16)
                nc.sync.dma_start(out=den128[:], in_=denrow[:])
                nc.vector.reciprocal(recip128[:], den128[:])
                nc.vector.tensor_copy(recip128h[:], recip128[:])
                nc.sync.dma_start(out=denrow_h[:], in_=recip128h[:])
                for ci, (n, h, qc) in enumerate(chunks):
                    hsl = slice(h * D, (h + 1) * D)
                    q0 = n * S + qc * QC
                    bc = psC.tile([128, QC], F32, tag=f"qk{h}", name="bc",
                                  bufs=1)
                    for j in range(QC // 512):
                        nc.tensor.matmul(
                            bc[0:D, j * 512:(j + 1) * 512],
                            ones_h[0:1, 0:D],
                            denrow_h[0:1, ci * QC + j * 512:ci * QC + (j + 1) * 512],
                            start=True, stop=True)
                    nc.vector.tensor_mul(attn_sb[hsl, q0:q0 + QC],
                                         attn_sb[hsl, q0:q0 + QC],
                                         bc[0:D, :])

            # ---- phase D: partial output projection out = attn^T.T @ woT ----
            with tc.tile_pool(name="psD", bufs=4, space="PSUM") as psD:
                for st in range(NS // 128):
                    op = psD.tile([128, E], F32, tag="op")
                    for j in range(E // 512):
                        nc.tensor.matmul(
                            op[:, j * 512:(j + 1) * 512],
                            attn_sb[:, st * 128:(st + 1) * 128],
                            wo_sb[:, j * 512:(j + 1) * 512],
                            start=True, stop=True)
                    ot = sbO.tile([128, E], F32, tag="ot", bufs=4)
                    # alternate copy + DMA engines to pipeline the drain
                    if st % 2 == 0:
                        nc.scalar.copy(ot[:], op[:])
                        nc.sync.dma_start(
                            out=out_d[st * 128:(st + 1) * 128, :], in_=ot[:])
                    else:
                        nc.vector.tensor_copy(ot[:], op[:])
                        nc.scalar.dma_start(
                            out=out_d[st * 128:(st + 1) * 128, :], in_=ot[:])

    _split_excess_waits(nc)
    return nc


_NC_CACHE = None


def _get_program():
    global _NC_CACHE
    if _NC_CACHE is None:
        _NC_CACHE = build_program()
    return _NC_CACHE


def _prep_inputs(x, mask, Wq, bq, Wk, bk, Wv, bv, Wo, bo):
    x = np.asarray(x, np.float32)
    xT = np.ascontiguousarray(x.reshape(NS, E).T.astype(np.float16))
    maskB = (np.asarray(mask, np.float32) - 1.0) * 30.0
    maskB = np.ascontiguousarray(
        maskB.reshape(NB, NKT, 128).transpose(2, 0, 1).reshape(128, NB * NKT))
    in_maps = []
    for i in range(N_CORES):
        c0 = i * C
        csl = slice(c0, c0 + C)
        in_maps.append({
            "xT": xT,
            "wq": np.ascontiguousarray(
                np.asarray(Wq, np.float32)[csl, :].T.astype(np.float16)),
            "wk": np.ascontiguousarray(
                np.asarray(Wk, np.float32)[csl, :].T.astype(np.float16)),
            "wv": np.ascontiguousarray(
                np.asarray(Wv, np.float32)[csl, :].T.astype(np.float16)),
            "wo": np.ascontiguousarray(
                np.asarray(Wo, np.float32)[:, csl].T.astype(np.float16)),
            "bq": np.asarray(bq, np.float32)[csl].reshape(C, 1).copy(),
            "bk": np.asarray(bk, np.float32)[csl].reshape(C, 1).copy(),
            "bv": np.asarray(bv, np.float32)[csl].reshape(C, 1).copy(),
            "maskB": maskB,
        })
    return in_maps


def run(trace=False, tmpdir=None, **inputs):
    """Run on hardware; returns (output, BassKernelResults)."""
    nc = _get_program()
    in_maps = _prep_inputs(**inputs)
    res = run_bass_kernel_spmd(nc, in_maps, list(range(N_CORES)), trace=trace,
                               tmpdir=tmpdir)
    partial = np.zeros((NS, E), np.float32)
    for i in range(N_CORES):
        partial += res.results[i]["out"]
    bo = np.asarray(inputs["bo"], np.float32)
    out = (partial + bo[None, :]).reshape(NB, S, E)
    return out, res


def kernel(**inputs):
    out, _ = run(trace=False, **inputs)
    return out
